# revision 1
# baseline (speedup 1.0000x reference)
"""EvoMultiheadSelfAttention Trainium2 kernel (8 NeuronCores, SPMD).

Sharding: core = (batch b, group of 4 heads). Data-parallel over B (2),
tensor-parallel over heads (16 -> 4 groups of 4). Each core computes its
4 heads' full+windowed attention and a partial output projection
out_part[e, t] = sum_{d in head-slice} Wo[e, d] * head_out[d, t].
Host sums the 4 partials per batch and adds bo.

Device-side layout notes:
  - Scores are computed transposed: sT[j, i] (keys on partitions) so the
    AV matmul (lhsT = v_aug [j, 65], rhs = pT [j, i]) accumulates
    oT [dh+1, i] with row 64 = softmax denominator (ones column trick).
  - The windowed attention reuses the same exp(s) values (window is a
    subset of the causal set); it only needs a multiplicative 0/1 mask.
  - 1/sqrt(dh) is folded into Wq/bq on the host.
"""

import numpy as np
import ml_dtypes

B, T, DM, H, WIN = 2, 2048, 1024, 16, 64
DH = DM // H          # 64
NCORES = 8
KS = DM // 128        # 8 d-subtiles
NT = T // 128         # 16 i/j tiles
NCH = T // 512        # 4 chunks of 512
BF16 = ml_dtypes.bfloat16

_CACHE: dict = {}


def _build_module():
    import os
    STAGE = int(os.environ.get("EVOATTN_STAGE", "9"))
    import contextlib
    import concourse.bass as bass  # noqa: F401
    import concourse.mybir as mybir
    import concourse.tile as tile
    from concourse import bacc
    from concourse.bass import ts

    f32 = mybir.dt.float32
    f32r = mybir.dt.float32r
    bf16 = mybir.dt.bfloat16
    EXP = mybir.ActivationFunctionType.Exp
    COPY = mybir.ActivationFunctionType.Identity
    MULT = mybir.AluOpType.mult
    ADD = mybir.AluOpType.add

    nc = bacc.Bacc("TRN2", target_bir_lowering=False, debug=False, num_devices=NCORES)

    def din(name, shape, dt):
        return nc.dram_tensor(name, shape, dt, kind="ExternalInput").ap()

    xb = din("xb", [T, DM], bf16)
    wq = din("wq", [2, 128, KS, 128], bf16)
    wk = din("wk", [2, 128, KS, 128], bf16)
    wv = din("wv", [2, 128, KS, 128], bf16)
    wo = din("wo", [128, 2, DM], bf16)
    bq = din("bq", [2, 128, 1], f32)
    bk = din("bk", [2, 128, 1], f32)
    bv = din("bv", [2, 128, 1], f32)
    mC = din("mC", [128, 128], bf16)       # additive causal mask (0 / -30000)
    wm = din("wm", [128, 256], bf16)       # 0/1 window masks [sub | diag]
    sg1 = din("sg1", [64, 1], f32)         # sigmoid(gate) per-partition scalar
    out = nc.dram_tensor("out", [DM, T], bf16, kind="ExternalOutput").ap()

    with tile.TileContext(nc) as tc:
        ctx = contextlib.ExitStack()
        consts = ctx.enter_context(tc.tile_pool(name="consts", bufs=1))
        big = ctx.enter_context(tc.tile_pool(name="big", bufs=1))
        pbulk = ctx.enter_context(tc.tile_pool(name="pbulk", bufs=40))
        pband = ctx.enter_context(tc.tile_pool(name="pband", bufs=10))
        ppw = ctx.enter_context(tc.tile_pool(name="ppw", bufs=10))
        rpool = ctx.enter_context(tc.tile_pool(name="rpool", bufs=3))
        npool = ctx.enter_context(tc.tile_pool(name="npool", bufs=2))
        opool = ctx.enter_context(tc.tile_pool(name="opool", bufs=4))
        psum_s = ctx.enter_context(tc.tile_pool(name="psum_s", bufs=4, space="PSUM"))
        psum_o = ctx.enter_context(tc.tile_pool(name="psum_o", bufs=4, space="PSUM"))
        dscr = ctx.enter_context(tc.tile_pool(name="dscr", bufs=4, space="DRAM"))

        # ---- constants into SBUF ----
        def cload(ap_in, shape, dt, tag):
            t_ = consts.tile(shape, dt, tag=tag, name=tag)
            nc.sync.dma_start(out=t_, in_=ap_in)
            return t_

        wq_sb = [cload(wq[p], [128, KS, 128], bf16, f"wq{p}") for p in (0, 1)]
        wk_sb = [cload(wk[p], [128, KS, 128], bf16, f"wk{p}") for p in (0, 1)]
        wv_sb = [cload(wv[p], [128, KS, 128], bf16, f"wv{p}") for p in (0, 1)]
        wo_sb = cload(wo, [128, 2, DM], bf16, "wo")
        bq_sb = [cload(bq[p], [128, 1], f32, f"bq{p}") for p in (0, 1)]
        bk_sb = [cload(bk[p], [128, 1], f32, f"bk{p}") for p in (0, 1)]
        bv_sb = [cload(bv[p], [128, 1], f32, f"bv{p}") for p in (0, 1)]
        mC_sb = cload(mC, [128, 128], bf16, "mC")
        wm_sb = cload(wm, [128, 256], bf16, "wm")
        sgc = cload(sg1, [64, 1], f32, "sgc")

        # ---- x transposed into SBUF: xT[dp, ks, t] ----
        xT = big.tile([128, KS, T], bf16, tag="xT", name="xT")
        for ks in range(KS):
            nc.sync.dma_start_transpose(xT[:, ks, :], xb[:, ts(ks, 128)])

        # ---- projections: qT/kT/vT pair tiles [2*64 dh, T] ----
        qT = [big.tile([128, T], bf16, tag=f"qT{p}", name=f"qT{p}") for p in (0, 1)]
        kT = [big.tile([128, T], bf16, tag=f"kT{p}", name=f"kT{p}") for p in (0, 1)]
        vT = [big.tile([128, T], bf16, tag=f"vT{p}", name=f"vT{p}") for p in (0, 1)]
        for p in (0, 1):
            for w_sb, b_sb, dst in ((wq_sb[p], bq_sb[p], qT[p]),
                                    (wk_sb[p], bk_sb[p], kT[p]),
                                    (wv_sb[p], bv_sb[p], vT[p])):
                for c4 in range(NCH):
                    ps = psum_s.tile([128, 512], f32, tag="ps", name="ps")
                    for ks in range(KS):
                        nc.tensor.matmul(ps, lhsT=w_sb[:, ks, :],
                                         rhs=xT[:, ks, ts(c4, 512)],
                                         start=(ks == 0), stop=(ks == KS - 1))
                    nc.scalar.activation(dst[:, ts(c4, 512)], ps, COPY, bias=b_sb)

        # ---- v transpose via DRAM bounce -> vh[h4][t_part, tt, 0:64] ----
        # (col 64 = ones for the denominator trick; stride padded to 128
        # elements because dma_start_transpose needs aligned dest offsets)
        vh = [big.tile([128, NT, 128], bf16, tag=f"vh{h4}", name=f"vh{h4}")
              for h4 in range(4)]
        for h4 in range(4):
            nc.vector.memset(vh[h4], 1.0)
        if STAGE >= 2:
            for p in (0, 1):
                vTd = dscr.tile([128, T], bf16, tag=f"vTd{p}", name=f"vTd{p}",
                                bufs=1)
                nc.sync.dma_start(out=vTd, in_=vT[p])
                for hh in (0, 1):
                    h4 = 2 * p + hh
                    for tt in range(NT):
                        nc.sync.dma_start_transpose(
                            vh[h4][:, tt, 0:64],
                            vTd[ts(hh, 64), ts(tt, 128)])

        # ---- attention ----
        oT_all = big.tile([128, 2, T], bf16, tag="oT_all", name="oT_all")
        if STAGE < 5:
            nc.vector.memset(oT_all, 0.0)
        for p in ((0, 1) if STAGE >= 3 else ()):
            for c in range(NCH if STAGE >= 5 else 2):
                nbulk = 4 * c
                pstore = {}
                # Phase A: scores + exp (+ window masks)
                for h in (0, 1):
                    hb = 64 * h
                    for jt in range(nbulk):
                        ps = psum_s.tile([128, 512], f32, tag="ps", name="ps")
                        nc.tensor.matmul(ps, lhsT=kT[p][hb:hb + 64, ts(jt, 128)],
                                         rhs=qT[p][hb:hb + 64, ts(c, 512)],
                                         start=True, stop=True)
                        pa = pbulk.tile([128, 512], bf16, tag="pa", name="pa")
                        nc.scalar.activation(pa, ps, EXP)
                        pstore[(h, jt)] = pa
                    for m in range(4):
                        t_ = 4 * c + m
                        bd = psum_s.tile([128, 512], f32, tag="ps", name="ps")
                        for mm in range(m + 1):
                            jt = 4 * c + mm
                            nc.tensor.matmul(bd[:, ts(mm, 128)],
                                             lhsT=kT[p][hb:hb + 64, ts(jt, 128)],
                                             rhs=qT[p][hb:hb + 64, ts(t_, 128)],
                                             start=True, stop=True,
                                             skip_group_check=True)
                        pb = pband.tile([128, 512], bf16, tag="pb", name="pb")
                        nc.scalar.activation(pb[:, 0:(m + 1) * 128],
                                             bd[:, 0:(m + 1) * 128], EXP)
                        nc.vector.tensor_tensor(pb[:, ts(m, 128)],
                                                pb[:, ts(m, 128)], mC_sb, MULT)
                        pstore[(h, "b", m)] = pb
                        pw = ppw.tile([128, 256], bf16, tag="pw", name="pw")
                        if t_ > 0:
                            sub_src = (pb[:, ts(m - 1, 128)] if m > 0
                                       else pstore[(h, nbulk - 1)][:, 0:128])
                            nc.vector.tensor_tensor(pw[:, 0:128], sub_src,
                                                    wm_sb[:, 0:128], MULT)
                        nc.vector.tensor_tensor(pw[:, 128:256], pb[:, ts(m, 128)],
                                                wm_sb[:, 128:256], MULT)
                        pstore[(h, "w", m)] = pw
                # Phase B: AV + normalization per head
                for h in ((0, 1) if STAGE >= 4 else ()):
                    h4 = 2 * p + h
                    OF = psum_o.tile([65, 512], f32, tag="oo", name="oo")
                    OW = psum_o.tile([65, 512], f32, tag="oo", name="oo")
                    for jt in range(nbulk):
                        nc.tensor.matmul(OF, lhsT=vh[h4][:, jt, 0:65],
                                         rhs=pstore[(h, jt)],
                                         start=(jt == 0), stop=False,
                                         skip_group_check=True)
                    for m in range(4):
                        t_ = 4 * c + m
                        pb = pstore[(h, "b", m)]
                        for mm in range(m + 1):
                            jt = 4 * c + mm
                            nc.tensor.matmul(OF[:, ts(m, 128)],
                                             lhsT=vh[h4][:, jt, 0:65],
                                             rhs=pb[:, ts(mm, 128)],
                                             start=(nbulk == 0 and mm == 0),
                                             stop=(m == 3 and mm == m),
                                             skip_group_check=True)
                        pw = pstore[(h, "w", m)]
                        if t_ > 0:
                            nc.tensor.matmul(OW[:, ts(m, 128)],
                                             lhsT=vh[h4][:, t_ - 1, 0:65],
                                             rhs=pw[:, 0:128],
                                             start=True, stop=False,
                                             skip_group_check=True)
                        nc.tensor.matmul(OW[:, ts(m, 128)],
                                         lhsT=vh[h4][:, t_, 0:65],
                                         rhs=pw[:, 128:256],
                                         start=(t_ == 0), stop=(m == 3),
                                         skip_group_check=True)
                    # l -> r (reciprocal of row 64), DRAM-bounce broadcast, combine
                    # (custom-DVE recip requires partition base 0: copy first)
                    lrow = rpool.tile([1, 1024], f32, tag="lrow", name="lrow")
                    nc.scalar.activation(lrow[:, 0:512], OF[64:65, :], COPY)
                    nc.vector.tensor_copy(lrow[:, 512:1024], OW[64:65, :])
                    rrow = rpool.tile([1, 1024], f32, tag="rrow", name="rrow")
                    nc.vector.reciprocal_approx_fast(rrow, lrow)
                    scr = dscr.tile([1, 1024], f32, tag="scr", name="scr")
                    nc.sync.dma_start(out=scr, in_=rrow)
                    rbf = npool.tile([64, 512], f32, tag="rbf", name="rbf")
                    rbw = npool.tile([64, 512], f32, tag="rbw", name="rbw")
                    nc.sync.dma_start(out=rbf,
                                      in_=scr[:, 0:512].to_broadcast([64, 512]))
                    nc.sync.dma_start(out=rbw,
                                      in_=scr[:, 512:1024].to_broadcast([64, 512]))
                    tf = npool.tile([64, 512], f32, tag="tf", name="tf")
                    tw = npool.tile([64, 512], f32, tag="tw", name="tw")
                    nc.vector.tensor_tensor(tf, OF[0:64, :], rbf, MULT)
                    nc.vector.scalar_tensor_tensor(tw, OW[0:64, :], sgc, rbw,
                                                   MULT, MULT)
                    oc = npool.tile([64, 512], bf16, tag="oc", name="oc")
                    nc.vector.tensor_tensor(oc, tf, tw, ADD)
                    # cross-partition placement into oT_all via DMA
                    nc.sync.dma_start(out=oT_all[ts(h, 64), p, ts(c, 512)], in_=oc)

        # ---- output projection: out[e, t] (partial over this core's d-slice) ----
        for et in range(8):
            pso = [psum_s.tile([128, 512], f32, tag="ps", name="ps") for _ in range(NCH)]
            for ks in (0, 1):
                for c4 in range(NCH):
                    nc.tensor.matmul(pso[c4], lhsT=wo_sb[:, ks, ts(et, 128)],
                                     rhs=oT_all[:, ks, ts(c4, 512)],
                                     start=(ks == 0), stop=(ks == 1))
            for c4 in range(NCH):
                ob = opool.tile([128, 512], bf16, tag="ob", name="ob")
                if c4 % 2 == 0:
                    nc.vector.tensor_copy(ob, pso[c4])
                else:
                    nc.scalar.activation(ob, pso[c4], COPY)
                nc.sync.dma_start(out=out[ts(et, 128), ts(c4, 512)], in_=ob)
        ctx.close()

    nc.compile()
    return nc


def _get_module():
    if "nc" not in _CACHE:
        _CACHE["nc"] = _build_module()
    return _CACHE["nc"]


def _prep_inputs(x, Wq, bq, Wk, bk, Wv, bv, Wo, bo, gate):
    """Build the 8 per-core input maps."""
    x = np.asarray(x, np.float32)
    Wq = np.asarray(Wq, np.float32)
    Wk = np.asarray(Wk, np.float32)
    Wv = np.asarray(Wv, np.float32)
    Wo = np.asarray(Wo, np.float32)
    bq = np.asarray(bq, np.float32)
    bk = np.asarray(bk, np.float32)
    bv = np.asarray(bv, np.float32)
    scale = 1.0 / np.sqrt(np.float32(DH))
    sg = float(1.0 / (1.0 + np.exp(-np.float32(gate))))

    j = np.arange(128)[:, None]
    i = np.arange(128)[None, :]
    mC = (j <= i).astype(BF16)
    wm_sub = (j >= i + 65).astype(BF16)
    wm_diag = ((j <= i) & (j >= i - 63)).astype(BF16)
    wm = np.concatenate([wm_sub, wm_diag], axis=1)
    sg1 = np.full((64, 1), sg, np.float32)

    def wslice(Wmat, e0, scl):
        # lhsT layout [dp, ks, e'] from W[e, d]: W.T restricted to e-slice
        Ws = (Wmat[e0:e0 + 128, :] * scl).astype(np.float32)  # [128 e', DM d]
        return np.ascontiguousarray(
            Ws.T.reshape(KS, 128, 128).transpose(1, 0, 2)).astype(BF16)

    in_maps = []
    for core in range(NCORES):
        b, g = divmod(core, 4)
        e0 = g * 256
        wq_c = np.stack([wslice(Wq, e0 + 128 * p, scale) for p in (0, 1)])
        wk_c = np.stack([wslice(Wk, e0 + 128 * p, 1.0) for p in (0, 1)])
        wv_c = np.stack([wslice(Wv, e0 + 128 * p, 1.0) for p in (0, 1)])
        # wo: [dp, ks, e] = Wo[e, e0 + ks*128 + dp]
        Wos = Wo[:, e0:e0 + 256]                        # [DM e, 256 d]
        wo_c = np.ascontiguousarray(
            Wos.T.reshape(2, 128, DM).transpose(1, 0, 2)).astype(BF16)
        in_maps.append({
            "xb": x[b].astype(BF16),
            "wq": wq_c, "wk": wk_c, "wv": wv_c, "wo": wo_c,
            "bq": (bq[e0:e0 + 256] * scale).reshape(2, 128, 1).astype(np.float32),
            "bk": bk[e0:e0 + 256].reshape(2, 128, 1).astype(np.float32),
            "bv": bv[e0:e0 + 256].reshape(2, 128, 1).astype(np.float32),
            "mC": mC, "wm": wm, "sg1": sg1,
        })
    return in_maps


def _run(nc, in_maps, **kw):
    from concourse.bass_utils import run_bass_kernel_spmd
    from concourse.bass_interp import get_hw_module
    old = nc.m
    nc.m = get_hw_module(nc.m)
    try:
        res = run_bass_kernel_spmd(nc, in_maps, core_ids=list(range(NCORES)), **kw)
    finally:
        nc.m = old
    return res


def kernel(x, Wq, bq, Wk, bk, Wv, bv, Wo, bo, gate):
    nc = _get_module()
    in_maps = _prep_inputs(x, Wq, bq, Wk, bk, Wv, bv, Wo, bo, gate)
    res = _run(nc, in_maps)
    bo = np.asarray(bo, np.float32)
    out = np.zeros((B, T, DM), np.float32)
    for core in range(NCORES):
        b = core // 4
        out[b] += res.results[core]["out"].astype(np.float32).T
    out += bo[None, None, :]
    return out

